# revision 17
# baseline (speedup 1.0000x reference)
"""Trainium2 Bass kernel for nn_BaselineModel_75256416960594 (retrieval_knn).

Computes, for feat_map (1,128,64,64) and feature_bank (50000,128):
    flat = l2_normalize(feat_map reshaped to (4096,128))
    d2[p,m] = ||flat_p||^2 + ||bank_m||^2 - 2 flat_p . bank_m
    patch_scores = sqrt(max(min_m d2, 0)) reshaped (64,64)
    anomaly_map = bilinear_upsample(patch_scores, 512, 512)  (half-pixel)
    anomaly_score = max(anomaly_map)

Sharding: feature_bank rows split across 8 NeuronCores (6250 rows each,
padded to 6272); patches replicated. Each core computes a partial min
over its shard; an AllReduce(min) combines partials; every core then
finishes the (tiny) sqrt/upsample/max tail identically.

Per-core dataflow (bank rows on partitions, patches on free dim), in
pairs of two 512-patch tiles ([128, 1024]) to amortize fixed costs.
Three per-pair pipelines, balanced across engines:
  - ACT-drain + DVE-min:  PE matmul -> ACT Identity(bias=b2) -> fp16 V ->
                          DVE tensor_tensor(min) at 2x fp16 mode
  - ACT-drain + Pool-min: same drain, min on GpSimd into a second buffer
  - direct:               b2 pre-accumulated in PSUM via K=1 ones-matmul,
                          DVE min straight from PSUM (f32)
Partition-axis min via PE transposes + DVE reduce_min, overlapped with the
next patch-pair's main loop. b2 (and its per-partition layout) is host prep.
"""

import functools

import numpy as np

import concourse.bacc as bacc
import concourse.mybir as mybir
import concourse.tile as tile
from concourse.bass_utils import run_bass_kernel_spmd
from concourse.masks import make_identity
from concourse import bass_isa

N_CORES = 8
C = 128            # feature channels
NPATCH = 4096      # 64*64 patches
HW = 64
OUT = 512
BANK = 50000
SHARD = BANK // N_CORES          # 6250
NQ = 49                          # bank chunks per core (49*128 = 6272)
PAD_SHARD = NQ * 128             # 6272
NT = NPATCH // 512               # 8 patch chunks of 512
NTP = 2                          # halves of 2048 patches ([128, 2048] tiles)
NBK = 7                          # bankT SBUF tiles of 896 cols (7 chunks each)
PAD_VAL = 15.5                   # pad rows: b2 = 128*15.5^2 = 30752 >> any real V
RINIT = -6.0e4                   # running-max init (negated-distance domain)

F16 = mybir.dt.float16
F32 = mybir.dt.float32


def _resize_matrix(out_size: int, in_size: int) -> np.ndarray:
    """Row-normalized triangle-kernel weights == jax.image.resize bilinear
    (half-pixel centers, upsampling)."""
    scale = in_size / out_size
    x = (np.arange(out_size) + 0.5) * scale - 0.5
    w = np.maximum(0.0, 1.0 - np.abs(x[:, None] - np.arange(in_size)[None, :]))
    w = w / w.sum(axis=1, keepdims=True)
    return w.astype(np.float32)


def _build(debug=False):
    nc = bacc.Bacc(num_devices=N_CORES)

    flatT = nc.dram_tensor("flatT", [C, NPATCH], F32, kind="ExternalInput")
    bankT = nc.dram_tensor("bankT", [C, PAD_SHARD], F16, kind="ExternalInput")
    b2h = nc.dram_tensor("b2h", [1, PAD_SHARD], F16, kind="ExternalInput")
    b2ppi = nc.dram_tensor("b2ppi", [128, NQ], F32, kind="ExternalInput")
    rni = nc.dram_tensor("rni", [1, NPATCH], F32, kind="ExternalInput")
    a2i = nc.dram_tensor("a2i", [HW, HW], F32, kind="ExternalInput")
    LT = nc.dram_tensor("LT", [HW, OUT], F32, kind="ExternalInput")
    out_map = nc.dram_tensor("out_map", [OUT, OUT], F32, kind="ExternalOutput")
    out_score = nc.dram_tensor("out_score", [1, 1], F32, kind="ExternalOutput")

    md_dram = nc.dram_tensor("md_dram", [1, NPATCH], F32, kind="Internal")
    md_red = nc.dram_tensor("md_red", [1, NPATCH], F32, kind="Internal",
                            addr_space="Shared")
    sc_dram = nc.dram_tensor("sc_dram", [128, 1], F32, kind="Internal")
    if debug:
        dbg_md = nc.dram_tensor("dbg_md", [128, 32], F32, kind="ExternalOutput")
        dbg_mdred = nc.dram_tensor("dbg_mdred", [128, 32], F32,
                                   kind="ExternalOutput")

    with tile.TileContext(nc) as tc:
        with tc.tile_pool(name="persist", bufs=1) as pp, \
             tc.tile_pool(name="vbuf", bufs=6) as vp, \
             tc.tile_pool(name="ps_main", bufs=2, space="PSUM") as psm:

            with nc.named_scope("setup"):
                # ---- inputs; separate tiles so consumers start per-chunk
                flatT_j = [pp.tile([C, 512], F32, name=f"flatT_{j}", tag=f"fl{j}")
                           for j in range(NT)]
                for j in range(NT):
                    nc.sync.dma_start(out=flatT_j[j],
                                      in_=flatT[:, j * 512:(j + 1) * 512])
                bankT_b = [pp.tile([C, 896], F16, name=f"bankT_{b}", tag=f"bk{b}")
                           for b in range(NBK)]
                for b in range(NBK):
                    nc.sync.dma_start(out=bankT_b[b],
                                      in_=bankT[:, b * 896:(b + 1) * 896])
                LT_sb = pp.tile([HW, OUT], F32)
                nc.sync.dma_start(out=LT_sb, in_=LT[:, :])
                b2h_sb = pp.tile([1, PAD_SHARD], F16)
                nc.sync.dma_start(out=b2h_sb, in_=b2h[:, :])
                b2pp = pp.tile([128, NQ], F32)
                nc.sync.dma_start(out=b2pp, in_=b2ppi[:, :])

                neg2_col = pp.tile([1, C], F32)
                nc.vector.memset(neg2_col, 2.0)
                ones_row16 = pp.tile([1, 512], F16)
                nc.vector.memset(ones_row16, 1.0)

                rn_sb = pp.tile([1, NPATCH], F32)
                nc.sync.dma_start(out=rn_sb, in_=rni[:, :])
                a2_64 = pp.tile([HW, HW], F32)
                nc.sync.dma_start(out=a2_64, in_=a2i[:, :])

                # flatT2 = (-2 * rn) broadcast * flatT   -> fp16, split per pair
                flatT2 = [pp.tile([C, 2048], F16, name=f"flatT2_{i}", tag=f"f2{i}")
                          for i in range(NTP)]
                for j in range(NT):
                    rb = psm.tile([128, 512], F32, tag="G")
                    nc.tensor.matmul(rb, neg2_col[:, :],
                                     rn_sb[:, j * 512:(j + 1) * 512],
                                     start=True, stop=True)
                    off = (j % 4) * 512
                    nc.vector.tensor_tensor(out=flatT2[j // 4][:, off:off + 512],
                                            in0=flatT_j[j], in1=rb,
                                            op=mybir.AluOpType.mult)


            # ---------------- main loop over (patch half, bank chunk)
            with nc.named_scope("main"):
                R = [pp.tile([128, 2048], F16, name=f"R_{i}", tag=f"R{i}")
                     for i in range(NTP)]
                for tp in range(NTP):
                    nc.vector.memset(R[tp], RINIT)
                for tp in range(NTP):
                    f2 = flatT2[tp]
                    for q in range(NQ):
                        bk = bankT_b[q // 7]
                        c0 = (q % 7) * 128
                        G = psm.tile([128, 2048], F32, tag="G")
                        for u in range(4):
                            nc.tensor.matmul(G[:, u * 512:(u + 1) * 512],
                                             bk[:, c0:c0 + 128],
                                             f2[:, u * 512:(u + 1) * 512],
                                             start=True, stop=True)
                        if q % 6 == 5:  # direct: DVE TS-add-b2 + TT max
                            V2 = vp.tile([128, 2048], F16, tag="V2", bufs=2)
                            nc.vector.tensor_scalar(
                                out=V2, in0=G, scalar1=b2pp[:, q:q + 1],
                                scalar2=None, op0=mybir.AluOpType.add)
                            nc.vector.tensor_tensor(out=R[tp], in0=R[tp],
                                                    in1=V2,
                                                    op=mybir.AluOpType.max)
                        else:           # ACT drain + DVE TT max
                            V = vp.tile([128, 2048], F16, tag="V")
                            nc.scalar.activation(
                                V, G, mybir.ActivationFunctionType.Identity,
                                bias=b2pp[:, q:q + 1], scale=1.0)
                            nc.vector.tensor_tensor(out=R[tp], in0=R[tp],
                                                    in1=V,
                                                    op=mybir.AluOpType.max)
                    # partition-axis max (GpSimd daisy-chain), 2 segments per
                    # half so the AllReduce overlaps the next half's compute
                    for sg in range(2):
                        par = vp.tile([128, 1024], F32, tag="par")
                        nc.gpsimd.partition_all_reduce(
                            par, R[tp][:, sg * 1024:(sg + 1) * 1024],
                            channels=128, reduce_op=bass_isa.ReduceOp.max)
                        so = tp * 2048 + sg * 1024
                        seg = md_dram[:, so:so + 1024]
                        nc.sync.dma_start(out=seg, in_=par[0:1, :])
                        nc.gpsimd.collective_compute(
                            "AllReduce", mybir.AluOpType.max,
                            replica_groups=[list(range(N_CORES))],
                            ins=[seg.opt()],
                            outs=[md_red[:, so:so + 1024].opt()])

            with nc.named_scope("post"):
                if debug:
                    mr_sb = pp.tile([128, 32], F32)
                    nc.sync.dma_start(
                        out=mr_sb,
                        in_=md_red[:, :].rearrange("o (t p) -> (o p) t", p=128))
                    nc.sync.dma_start(out=dbg_mdred[:, :], in_=mr_sb)

                # ---- tail: d = sqrt(max(a2 + min_d2, 0)), upsample, max
                S2 = pp.tile([HW, HW], F32)
                nc.sync.dma_start(
                    out=S2,
                    in_=md_red[:, :].rearrange("o (h w) -> (o h) w", w=HW))
                nc.vector.tensor_tensor(out=S2, in0=a2_64, in1=S2,
                                        op=mybir.AluOpType.subtract)
                nc.vector.tensor_scalar_max(S2, S2, 0.0)
                # sqrt + one Newton step (ACT sqrt table budget is loose)
                S = pp.tile([HW, HW], F32)
                nc.scalar.activation(S, S2, mybir.ActivationFunctionType.Sqrt)

                # transpose S for the first upsample matmul
                ident32 = pp.tile([HW, HW], F32)
                make_identity(nc, ident32)
                St_ps = psm.tile([HW, HW], F32, tag="G")
                nc.tensor.transpose(St_ps, S[:, :], ident32[:, :])
                St = pp.tile([HW, HW], F32)
                nc.scalar.copy(St, St_ps)
                # A = S @ L^T  [64, 512]
                A_ps = psm.tile([HW, OUT], F32, tag="G")
                nc.tensor.matmul(A_ps, St[:, :], LT_sb[:, :], start=True, stop=True)
                A_sb = pp.tile([HW, OUT], F32)
                nc.scalar.copy(A_sb, A_ps)
                # U = L @ A  [512, 512], in 4 chunks of 128 rows
                rmax = pp.tile([128, 4], F32)
                for c4 in range(4):
                    U_ps = psm.tile([128, OUT], F32, tag="G")
                    nc.tensor.matmul(U_ps, LT_sb[:, c4 * 128:(c4 + 1) * 128],
                                     A_sb[:, :], start=True, stop=True)
                    U_c = vp.tile([128, OUT], F32, tag="U")
                    nc.scalar.copy(U_c, U_ps)
                    nc.sync.dma_start(
                        out=out_map[c4 * 128:(c4 + 1) * 128, :], in_=U_c)
                    nc.vector.tensor_reduce(out=rmax[:, c4:c4 + 1], in_=U_c,
                                            axis=mybir.AxisListType.X,
                                            op=mybir.AluOpType.max)

                # anomaly_score = max over the upsampled map
                rmax1 = pp.tile([128, 1], F32)
                nc.vector.tensor_reduce(out=rmax1, in_=rmax,
                                        axis=mybir.AxisListType.X,
                                        op=mybir.AluOpType.max)
                nc.sync.dma_start(out=sc_dram[:, :], in_=rmax1)
                rrow = pp.tile([1, 128], F32)
                nc.sync.dma_start(out=rrow,
                                  in_=sc_dram[:, :].rearrange("p a -> a p"))
                s11 = pp.tile([1, 1], F32)
                nc.vector.tensor_reduce(out=s11, in_=rrow,
                                        axis=mybir.AxisListType.X,
                                        op=mybir.AluOpType.max)
                nc.sync.dma_start(out=out_score[:, :], in_=s11)

    nc.finalize()
    return nc


@functools.lru_cache(maxsize=2)
def _get_nc(debug=False):
    return _build(debug)


def _prep_in_maps(feat_map, feature_bank):
    flatT_np = np.ascontiguousarray(feat_map.reshape(C, NPATCH))
    LT_np = np.ascontiguousarray(_resize_matrix(OUT, HW).T)  # [64, 512]
    nsq = (flatT_np.astype(np.float64) ** 2).sum(axis=0)
    rn = (1.0 / np.maximum(np.sqrt(nsq), 1e-12))
    a2 = (nsq * rn * rn).astype(np.float32)
    rn_np = rn.astype(np.float32).reshape(1, NPATCH)
    a2_np = np.ascontiguousarray(a2.reshape(HW, HW))
    in_maps = []
    for c in range(N_CORES):
        shard = feature_bank[c * SHARD:(c + 1) * SHARD]          # [6250, 128]
        pad = np.full((PAD_SHARD - SHARD, C), PAD_VAL, np.float32)
        shard = np.concatenate([shard, pad], axis=0)             # [6272, 128]
        shard16 = shard.astype(np.float16)
        bankT_np = np.ascontiguousarray(shard16.T)               # [128, 6272]
        b2 = (shard16.astype(np.float32) ** 2).sum(axis=1)       # [6272]
        b2h_np = (-b2).astype(np.float16).reshape(1, PAD_SHARD)
        b2pp_np = np.ascontiguousarray((-b2).reshape(NQ, 128).T.astype(np.float32))
        in_maps.append({"flatT": flatT_np, "bankT": bankT_np,
                        "b2h": b2h_np, "b2ppi": b2pp_np, "LT": LT_np,
                        "rni": rn_np, "a2i": a2_np})
    return in_maps


def kernel(feat_map, feature_bank, out_size):
    assert int(out_size) == OUT, f"kernel hardcodes out_size={OUT}"
    feat_map = np.asarray(feat_map, dtype=np.float32)
    feature_bank = np.asarray(feature_bank, dtype=np.float32)
    assert feat_map.shape == (1, C, HW, HW)
    assert feature_bank.shape == (BANK, C)

    nc = _get_nc()
    res = run_bass_kernel_spmd(nc, _prep_in_maps(feat_map, feature_bank),
                               core_ids=list(range(N_CORES)))
    r0 = res.results[0]
    anomaly_map = np.asarray(r0["out_map"], dtype=np.float32)
    anomaly_score = np.float32(np.asarray(r0["out_score"]).reshape(()))
    return anomaly_map, anomaly_score


# revision 18
# speedup vs baseline: 1.0883x; 1.0883x over previous
"""Trainium2 Bass kernel for nn_BaselineModel_75256416960594 (retrieval_knn).

Computes, for feat_map (1,128,64,64) and feature_bank (50000,128):
    flat = l2_normalize(feat_map reshaped to (4096,128))
    d2[p,m] = ||flat_p||^2 + ||bank_m||^2 - 2 flat_p . bank_m
    patch_scores = sqrt(max(min_m d2, 0)) reshaped (64,64)
    anomaly_map = bilinear_upsample(patch_scores, 512, 512)  (half-pixel)
    anomaly_score = max(anomaly_map)

Sharding: feature_bank rows split across 8 NeuronCores (6250 rows each,
padded to 6272); patches replicated. Each core computes a partial min
over its shard; an AllReduce(min) combines partials; every core then
finishes the (tiny) sqrt/upsample/max tail identically.

Per-core dataflow (bank rows on partitions, patches on free dim), in
pairs of two 512-patch tiles ([128, 1024]) to amortize fixed costs.
Three per-pair pipelines, balanced across engines:
  - ACT-drain + DVE-min:  PE matmul -> ACT Identity(bias=b2) -> fp16 V ->
                          DVE tensor_tensor(min) at 2x fp16 mode
  - ACT-drain + Pool-min: same drain, min on GpSimd into a second buffer
  - direct:               b2 pre-accumulated in PSUM via K=1 ones-matmul,
                          DVE min straight from PSUM (f32)
Partition-axis min via PE transposes + DVE reduce_min, overlapped with the
next patch-pair's main loop. b2 (and its per-partition layout) is host prep.
"""

import functools

import numpy as np

import concourse.bacc as bacc
import concourse.mybir as mybir
import concourse.tile as tile
from concourse.bass_utils import run_bass_kernel_spmd
from concourse.masks import make_identity
from concourse import bass_isa

N_CORES = 8
C = 128            # feature channels
NPATCH = 4096      # 64*64 patches
HW = 64
OUT = 512
BANK = 50000
SHARD = BANK // N_CORES          # 6250
NQ = 49                          # bank chunks per core (49*128 = 6272)
PAD_SHARD = NQ * 128             # 6272
NT = NPATCH // 512               # 8 patch chunks of 512
NTP = 2                          # halves of 2048 patches ([128, 2048] tiles)
NBK = 7                          # bankT SBUF tiles of 896 cols (7 chunks each)
PAD_VAL = 15.5                   # pad rows: b2 = 128*15.5^2 = 30752 >> any real V
RINIT = -6.0e4                   # running-max init (negated-distance domain)

F16 = mybir.dt.float16
F32 = mybir.dt.float32


def _resize_matrix(out_size: int, in_size: int) -> np.ndarray:
    """Row-normalized triangle-kernel weights == jax.image.resize bilinear
    (half-pixel centers, upsampling)."""
    scale = in_size / out_size
    x = (np.arange(out_size) + 0.5) * scale - 0.5
    w = np.maximum(0.0, 1.0 - np.abs(x[:, None] - np.arange(in_size)[None, :]))
    w = w / w.sum(axis=1, keepdims=True)
    return w.astype(np.float32)


def _build(debug=False):
    nc = bacc.Bacc(num_devices=N_CORES)

    flatT = nc.dram_tensor("flatT", [C, NPATCH], F32, kind="ExternalInput")
    bankT = nc.dram_tensor("bankT", [C, PAD_SHARD], F16, kind="ExternalInput")
    b2h = nc.dram_tensor("b2h", [1, PAD_SHARD], F16, kind="ExternalInput")
    b2ppi = nc.dram_tensor("b2ppi", [128, NQ], F32, kind="ExternalInput")
    rni = nc.dram_tensor("rni", [1, NPATCH], F32, kind="ExternalInput")
    a2i = nc.dram_tensor("a2i", [HW, HW], F32, kind="ExternalInput")
    LT = nc.dram_tensor("LT", [HW, OUT], F32, kind="ExternalInput")
    out_map = nc.dram_tensor("out_map", [OUT, OUT], F32, kind="ExternalOutput")
    out_score = nc.dram_tensor("out_score", [1, 1], F32, kind="ExternalOutput")

    md_dram = nc.dram_tensor("md_dram", [1, NPATCH], F32, kind="Internal")
    md_red = nc.dram_tensor("md_red", [1, NPATCH], F32, kind="Internal",
                            addr_space="Shared")
    sc_dram = nc.dram_tensor("sc_dram", [128, 1], F32, kind="Internal")
    if debug:
        dbg_md = nc.dram_tensor("dbg_md", [128, 32], F32, kind="ExternalOutput")
        dbg_mdred = nc.dram_tensor("dbg_mdred", [128, 32], F32,
                                   kind="ExternalOutput")

    with tile.TileContext(nc) as tc:
        with tc.tile_pool(name="persist", bufs=1) as pp, \
             tc.tile_pool(name="vbuf", bufs=6) as vp, \
             tc.tile_pool(name="ps_main", bufs=2, space="PSUM") as psm:

            with nc.named_scope("setup"):
                # ---- inputs; separate tiles so consumers start per-chunk
                flatT_j = [pp.tile([C, 512], F32, name=f"flatT_{j}", tag=f"fl{j}")
                           for j in range(NT)]
                for j in range(NT):
                    nc.sync.dma_start(out=flatT_j[j],
                                      in_=flatT[:, j * 512:(j + 1) * 512])
                bankT_b = [pp.tile([C, 896], F16, name=f"bankT_{b}", tag=f"bk{b}")
                           for b in range(NBK)]
                for b in range(NBK):
                    nc.sync.dma_start(out=bankT_b[b],
                                      in_=bankT[:, b * 896:(b + 1) * 896])
                LT_sb = pp.tile([HW, OUT], F32)
                nc.sync.dma_start(out=LT_sb, in_=LT[:, :])
                b2h_sb = pp.tile([1, PAD_SHARD], F16)
                nc.sync.dma_start(out=b2h_sb, in_=b2h[:, :])
                b2pp = pp.tile([128, NQ], F32)
                nc.sync.dma_start(out=b2pp, in_=b2ppi[:, :])

                neg2_col = pp.tile([1, C], F32)
                nc.vector.memset(neg2_col, 2.0)
                ones_row16 = pp.tile([1, 512], F16)
                nc.vector.memset(ones_row16, 1.0)

                rn_sb = pp.tile([1, NPATCH], F32)
                nc.sync.dma_start(out=rn_sb, in_=rni[:, :])
                a2_64 = pp.tile([HW, HW], F32)
                nc.sync.dma_start(out=a2_64, in_=a2i[:, :])

                # flatT2 = (-2 * rn) broadcast * flatT   -> fp16, split per pair
                flatT2 = [pp.tile([C, 2048], F16, name=f"flatT2_{i}", tag=f"f2{i}")
                          for i in range(NTP)]
                for j in range(NT):
                    rb = psm.tile([128, 512], F32, tag="G")
                    nc.tensor.matmul(rb, neg2_col[:, :],
                                     rn_sb[:, j * 512:(j + 1) * 512],
                                     start=True, stop=True)
                    off = (j % 4) * 512
                    nc.vector.tensor_tensor(out=flatT2[j // 4][:, off:off + 512],
                                            in0=flatT_j[j], in1=rb,
                                            op=mybir.AluOpType.mult)


            # ---------------- main loop over (patch half, bank chunk)
            with nc.named_scope("main"):
                R = [pp.tile([128, 2048], F16, name=f"R_{i}", tag=f"R{i}")
                     for i in range(NTP)]
                for tp in range(NTP):
                    nc.vector.memset(R[tp], RINIT)
                for tp in range(NTP):
                    f2 = flatT2[tp]
                    for q in range(NQ):
                        bk = bankT_b[q // 7]
                        c0 = (q % 7) * 128
                        G = psm.tile([128, 2048], F32, tag="G")
                        for u in range(4):
                            nc.tensor.matmul(G[:, u * 512:(u + 1) * 512],
                                             bk[:, c0:c0 + 128],
                                             f2[:, u * 512:(u + 1) * 512],
                                             start=True, stop=True)
                        if q % 4 == 3:  # direct: one DVE STT: R = max(G+b2, R)
                            nc.vector.scalar_tensor_tensor(
                                out=R[tp], in0=G, scalar=b2pp[:, q:q + 1],
                                in1=R[tp], op0=mybir.AluOpType.add,
                                op1=mybir.AluOpType.max)
                        else:           # ACT drain + DVE TT max
                            V = vp.tile([128, 2048], F16, tag="V")
                            nc.scalar.activation(
                                V, G, mybir.ActivationFunctionType.Identity,
                                bias=b2pp[:, q:q + 1], scale=1.0)
                            nc.vector.tensor_tensor(out=R[tp], in0=R[tp],
                                                    in1=V,
                                                    op=mybir.AluOpType.max)
                    # partition-axis max (GpSimd daisy-chain), 2 segments per
                    # half so the AllReduce overlaps the next half's compute
                    for sg in range(2):
                        par = vp.tile([128, 1024], F32, tag="par")
                        nc.gpsimd.partition_all_reduce(
                            par, R[tp][:, sg * 1024:(sg + 1) * 1024],
                            channels=128, reduce_op=bass_isa.ReduceOp.max)
                        so = tp * 2048 + sg * 1024
                        seg = md_dram[:, so:so + 1024]
                        nc.sync.dma_start(out=seg, in_=par[0:1, :])
                        nc.gpsimd.collective_compute(
                            "AllReduce", mybir.AluOpType.max,
                            replica_groups=[list(range(N_CORES))],
                            ins=[seg.opt()],
                            outs=[md_red[:, so:so + 1024].opt()])

            with nc.named_scope("post"):
                if debug:
                    mr_sb = pp.tile([128, 32], F32)
                    nc.sync.dma_start(
                        out=mr_sb,
                        in_=md_red[:, :].rearrange("o (t p) -> (o p) t", p=128))
                    nc.sync.dma_start(out=dbg_mdred[:, :], in_=mr_sb)

                # ---- tail: d = sqrt(max(a2 + min_d2, 0)) computed directly in
                # transposed [w, h] layout (St = S^T), feeding the upsample
                # matmul without a PE transpose. a2i arrives transposed.
                S2 = pp.tile([HW, HW], F32)
                nc.sync.dma_start(
                    out=S2,
                    in_=md_red[:, :].rearrange("o (h w) -> (o w) h", w=HW))
                nc.vector.tensor_tensor(out=S2, in0=a2_64, in1=S2,
                                        op=mybir.AluOpType.subtract)
                nc.vector.tensor_scalar_max(S2, S2, 0.0)
                St = pp.tile([HW, HW], F32)
                nc.scalar.activation(St, S2, mybir.ActivationFunctionType.Sqrt)
                # A = S @ L^T  [64, 512]
                A_ps = psm.tile([HW, OUT], F32, tag="G")
                nc.tensor.matmul(A_ps, St[:, :], LT_sb[:, :], start=True, stop=True)
                A_sb = pp.tile([HW, OUT], F32)
                nc.scalar.copy(A_sb, A_ps)
                # U = L @ A  [512, 512], in 4 chunks of 128 rows
                rmax = pp.tile([128, 4], F32)
                for c4 in range(4):
                    U_ps = psm.tile([128, OUT], F32, tag="G")
                    nc.tensor.matmul(U_ps, LT_sb[:, c4 * 128:(c4 + 1) * 128],
                                     A_sb[:, :], start=True, stop=True)
                    U_c = vp.tile([128, OUT], F32, tag="U")
                    nc.scalar.copy(U_c, U_ps)
                    nc.sync.dma_start(
                        out=out_map[c4 * 128:(c4 + 1) * 128, :], in_=U_c)
                    nc.vector.tensor_reduce(out=rmax[:, c4:c4 + 1], in_=U_c,
                                            axis=mybir.AxisListType.X,
                                            op=mybir.AluOpType.max)

                # anomaly_score = max over the upsampled map
                rmax1 = pp.tile([128, 1], F32)
                nc.vector.tensor_reduce(out=rmax1, in_=rmax,
                                        axis=mybir.AxisListType.X,
                                        op=mybir.AluOpType.max)
                nc.sync.dma_start(out=sc_dram[:, :], in_=rmax1)
                rrow = pp.tile([1, 128], F32)
                nc.sync.dma_start(out=rrow,
                                  in_=sc_dram[:, :].rearrange("p a -> a p"))
                s11 = pp.tile([1, 1], F32)
                nc.vector.tensor_reduce(out=s11, in_=rrow,
                                        axis=mybir.AxisListType.X,
                                        op=mybir.AluOpType.max)
                nc.sync.dma_start(out=out_score[:, :], in_=s11)

    nc.finalize()
    return nc


@functools.lru_cache(maxsize=2)
def _get_nc(debug=False):
    return _build(debug)


def _prep_in_maps(feat_map, feature_bank):
    flatT_np = np.ascontiguousarray(feat_map.reshape(C, NPATCH))
    LT_np = np.ascontiguousarray(_resize_matrix(OUT, HW).T)  # [64, 512]
    nsq = (flatT_np.astype(np.float64) ** 2).sum(axis=0)
    rn = (1.0 / np.maximum(np.sqrt(nsq), 1e-12))
    a2 = (nsq * rn * rn).astype(np.float32)
    rn_np = rn.astype(np.float32).reshape(1, NPATCH)
    a2_np = np.ascontiguousarray(a2.reshape(HW, HW).T)
    in_maps = []
    for c in range(N_CORES):
        shard = feature_bank[c * SHARD:(c + 1) * SHARD]          # [6250, 128]
        pad = np.full((PAD_SHARD - SHARD, C), PAD_VAL, np.float32)
        shard = np.concatenate([shard, pad], axis=0)             # [6272, 128]
        shard16 = shard.astype(np.float16)
        bankT_np = np.ascontiguousarray(shard16.T)               # [128, 6272]
        b2 = (shard16.astype(np.float32) ** 2).sum(axis=1)       # [6272]
        b2h_np = (-b2).astype(np.float16).reshape(1, PAD_SHARD)
        b2pp_np = np.ascontiguousarray((-b2).reshape(NQ, 128).T.astype(np.float32))
        in_maps.append({"flatT": flatT_np, "bankT": bankT_np,
                        "b2h": b2h_np, "b2ppi": b2pp_np, "LT": LT_np,
                        "rni": rn_np, "a2i": a2_np})
    return in_maps


def kernel(feat_map, feature_bank, out_size):
    assert int(out_size) == OUT, f"kernel hardcodes out_size={OUT}"
    feat_map = np.asarray(feat_map, dtype=np.float32)
    feature_bank = np.asarray(feature_bank, dtype=np.float32)
    assert feat_map.shape == (1, C, HW, HW)
    assert feature_bank.shape == (BANK, C)

    nc = _get_nc()
    res = run_bass_kernel_spmd(nc, _prep_in_maps(feat_map, feature_bank),
                               core_ids=list(range(N_CORES)))
    r0 = res.results[0]
    anomaly_map = np.asarray(r0["out_map"], dtype=np.float32)
    anomaly_score = np.float32(np.asarray(r0["out_score"]).reshape(()))
    return anomaly_map, anomaly_score


# revision 19
# speedup vs baseline: 1.3594x; 1.2491x over previous
"""Trainium2 Bass kernel for nn_BaselineModel_75256416960594 (retrieval_knn).

Computes, for feat_map (1,128,64,64) and feature_bank (50000,128):
    flat = l2_normalize(feat_map reshaped to (4096,128))
    d2[p,m] = ||flat_p||^2 + ||bank_m||^2 - 2 flat_p . bank_m
    patch_scores = sqrt(max(min_m d2, 0)) reshaped (64,64)
    anomaly_map = bilinear_upsample(patch_scores, 512, 512)  (half-pixel)
    anomaly_score = max(anomaly_map)

Sharding: feature_bank rows split across 8 NeuronCores (6250 rows each,
padded to 6272); patches replicated. Each core computes a partial min
over its shard; an AllReduce(min) combines partials; every core then
finishes the (tiny) sqrt/upsample/max tail identically.

Per-core dataflow (bank rows on partitions, patches on free dim), in
pairs of two 512-patch tiles ([128, 1024]) to amortize fixed costs.
Three per-pair pipelines, balanced across engines:
  - ACT-drain + DVE-min:  PE matmul -> ACT Identity(bias=b2) -> fp16 V ->
                          DVE tensor_tensor(min) at 2x fp16 mode
  - ACT-drain + Pool-min: same drain, min on GpSimd into a second buffer
  - direct:               b2 pre-accumulated in PSUM via K=1 ones-matmul,
                          DVE min straight from PSUM (f32)
Partition-axis min via PE transposes + DVE reduce_min, overlapped with the
next patch-pair's main loop. b2 (and its per-partition layout) is host prep.
"""

import functools

import numpy as np

import concourse.bacc as bacc
import concourse.mybir as mybir
import concourse.tile as tile
from concourse.bass_utils import run_bass_kernel_spmd
from concourse.masks import make_identity
from concourse import bass_isa

N_CORES = 8
C = 128            # feature channels
NPATCH = 4096      # 64*64 patches
HW = 64
OUT = 512
BANK = 50000
SHARD = BANK // N_CORES          # 6250
NQ = 49                          # bank chunks per core (49*128 = 6272)
PAD_SHARD = NQ * 128             # 6272
NT = NPATCH // 512               # 8 patch chunks of 512
NTP = 2                          # halves of 2048 patches ([128, 2048] tiles)
NBK = 7                          # bankT SBUF tiles of 896 cols (7 chunks each)
PAD_VAL = 15.5                   # pad rows: b2 = 128*15.5^2 = 30752 >> any real V
RINIT = -6.0e4                   # running-max init (negated-distance domain)

F16 = mybir.dt.float16
F32 = mybir.dt.float32


def _resize_matrix(out_size: int, in_size: int) -> np.ndarray:
    """Row-normalized triangle-kernel weights == jax.image.resize bilinear
    (half-pixel centers, upsampling)."""
    scale = in_size / out_size
    x = (np.arange(out_size) + 0.5) * scale - 0.5
    w = np.maximum(0.0, 1.0 - np.abs(x[:, None] - np.arange(in_size)[None, :]))
    w = w / w.sum(axis=1, keepdims=True)
    return w.astype(np.float32)


def _build(debug=False):
    nc = bacc.Bacc(num_devices=N_CORES)

    flatT = nc.dram_tensor("flatT", [C, NPATCH], F32, kind="ExternalInput")
    bankT = nc.dram_tensor("bankT", [C, PAD_SHARD], F16, kind="ExternalInput")
    b2h = nc.dram_tensor("b2h", [1, PAD_SHARD], F16, kind="ExternalInput")
    b2ppi = nc.dram_tensor("b2ppi", [128, NQ], F32, kind="ExternalInput")
    rni = nc.dram_tensor("rni", [1, NPATCH], F32, kind="ExternalInput")
    a2i = nc.dram_tensor("a2i", [HW, HW], F32, kind="ExternalInput")
    LT = nc.dram_tensor("LT", [HW, OUT], F32, kind="ExternalInput")
    out_map = nc.dram_tensor("out_map", [OUT, OUT], F32, kind="ExternalOutput")
    out_score = nc.dram_tensor("out_score", [1, 1], F32, kind="ExternalOutput")

    md_dram = nc.dram_tensor("md_dram", [1, NPATCH], F32, kind="Internal")
    md_red = nc.dram_tensor("md_red", [1, NPATCH], F32, kind="Internal",
                            addr_space="Shared")
    sc_dram = nc.dram_tensor("sc_dram", [128, 1], F32, kind="Internal")
    if debug:
        dbg_md = nc.dram_tensor("dbg_md", [128, 32], F32, kind="ExternalOutput")
        dbg_mdred = nc.dram_tensor("dbg_mdred", [128, 32], F32,
                                   kind="ExternalOutput")

    with tile.TileContext(nc) as tc:
        with tc.tile_pool(name="persist", bufs=1) as pp, \
             tc.tile_pool(name="vbuf", bufs=8) as vp, \
             tc.tile_pool(name="ps_main", bufs=4, space="PSUM") as psm:

            with nc.named_scope("setup"):
                # ---- inputs; separate tiles so consumers start per-chunk
                flatT_j = [pp.tile([C, 512], F32, name=f"flatT_{j}", tag=f"fl{j}")
                           for j in range(NT)]
                for j in range(NT):
                    nc.sync.dma_start(out=flatT_j[j],
                                      in_=flatT[:, j * 512:(j + 1) * 512])
                bankT_b = [pp.tile([C, 896], F16, name=f"bankT_{b}", tag=f"bk{b}")
                           for b in range(NBK)]
                for b in range(NBK):
                    nc.sync.dma_start(out=bankT_b[b],
                                      in_=bankT[:, b * 896:(b + 1) * 896])
                LT_sb = pp.tile([HW, OUT], F32)
                nc.sync.dma_start(out=LT_sb, in_=LT[:, :])
                b2h_sb = pp.tile([1, PAD_SHARD], F16)
                nc.sync.dma_start(out=b2h_sb, in_=b2h[:, :])
                b2pp = pp.tile([128, NQ], F32)
                nc.sync.dma_start(out=b2pp, in_=b2ppi[:, :])

                neg2_col = pp.tile([1, C], F32)
                nc.vector.memset(neg2_col, 2.0)
                ones_row16 = pp.tile([1, 512], F16)
                nc.vector.memset(ones_row16, 1.0)

                rn_sb = pp.tile([1, NPATCH], F32)
                nc.sync.dma_start(out=rn_sb, in_=rni[:, :])
                a2_64 = pp.tile([HW, HW], F32)
                nc.sync.dma_start(out=a2_64, in_=a2i[:, :])

                # flatT2 = (-2 * rn) broadcast * flatT   -> fp16, split per pair
                flatT2 = [pp.tile([C, 2048], F16, name=f"flatT2_{i}", tag=f"f2{i}")
                          for i in range(NTP)]
                for j in range(NT):
                    rb = psm.tile([128, 512], F32, tag="G")
                    nc.tensor.matmul(rb, neg2_col[:, :],
                                     rn_sb[:, j * 512:(j + 1) * 512],
                                     start=True, stop=True)
                    off = (j % 4) * 512
                    nc.vector.tensor_tensor(out=flatT2[j // 4][:, off:off + 512],
                                            in0=flatT_j[j], in1=rb,
                                            op=mybir.AluOpType.mult)


            # ---------------- main loop over (patch half, bank chunk)
            with nc.named_scope("main"):
                R = [pp.tile([128, 2048], F16, name=f"R_{i}", tag=f"R{i}")
                     for i in range(NTP)]
                for tp in range(NTP):
                    nc.vector.memset(R[tp], RINIT)
                for tp in range(NTP):
                    f2 = flatT2[tp]
                    for q in range(NQ):
                        bk = bankT_b[q // 7]
                        c0 = (q % 7) * 128
                        for u in range(2):
                            Ro = R[tp][:, u * 1024:(u + 1) * 1024]
                            G = psm.tile([128, 1024], F32, tag="G")
                            nc.tensor.matmul(G[:, 0:512], bk[:, c0:c0 + 128],
                                             f2[:, u * 1024:u * 1024 + 512],
                                             start=True, stop=True)
                            nc.tensor.matmul(G[:, 512:1024], bk[:, c0:c0 + 128],
                                             f2[:, u * 1024 + 512:(u + 1) * 1024],
                                             start=True, stop=True)
                            if q % 4 == 3:  # direct: DVE STT: R = max(G+b2, R)
                                nc.vector.scalar_tensor_tensor(
                                    out=Ro, in0=G, scalar=b2pp[:, q:q + 1],
                                    in1=Ro, op0=mybir.AluOpType.add,
                                    op1=mybir.AluOpType.max)
                            else:           # ACT drain + DVE TT max
                                V = vp.tile([128, 1024], F16, tag="V")
                                nc.scalar.activation(
                                    V, G,
                                    mybir.ActivationFunctionType.Identity,
                                    bias=b2pp[:, q:q + 1], scale=1.0)
                                nc.vector.tensor_tensor(out=Ro, in0=Ro, in1=V,
                                                        op=mybir.AluOpType.max)
                    # partition-axis max (GpSimd daisy-chain), 2 segments per
                    # half so the AllReduce overlaps the next half's compute
                    for sg in range(2):
                        par = vp.tile([128, 1024], F32, tag="par")
                        nc.gpsimd.partition_all_reduce(
                            par, R[tp][:, sg * 1024:(sg + 1) * 1024],
                            channels=128, reduce_op=bass_isa.ReduceOp.max)
                        so = tp * 2048 + sg * 1024
                        seg = md_dram[:, so:so + 1024]
                        nc.sync.dma_start(out=seg, in_=par[0:1, :])
                        nc.gpsimd.collective_compute(
                            "AllReduce", mybir.AluOpType.max,
                            replica_groups=[list(range(N_CORES))],
                            ins=[seg.opt()],
                            outs=[md_red[:, so:so + 1024].opt()])

            with nc.named_scope("post"):
                if debug:
                    mr_sb = pp.tile([128, 32], F32)
                    nc.sync.dma_start(
                        out=mr_sb,
                        in_=md_red[:, :].rearrange("o (t p) -> (o p) t", p=128))
                    nc.sync.dma_start(out=dbg_mdred[:, :], in_=mr_sb)

                # ---- tail: d = sqrt(max(a2 + min_d2, 0)) computed directly in
                # transposed [w, h] layout (St = S^T), feeding the upsample
                # matmul without a PE transpose. a2i arrives transposed.
                S2 = pp.tile([HW, HW], F32)
                nc.sync.dma_start(
                    out=S2,
                    in_=md_red[:, :].rearrange("o (h w) -> (o w) h", w=HW))
                nc.vector.tensor_tensor(out=S2, in0=a2_64, in1=S2,
                                        op=mybir.AluOpType.subtract)
                nc.vector.tensor_scalar_max(S2, S2, 0.0)
                St = pp.tile([HW, HW], F32)
                nc.scalar.activation(St, S2, mybir.ActivationFunctionType.Sqrt)
                # A = S @ L^T  [64, 512]
                A_ps = psm.tile([HW, OUT], F32, tag="G")
                nc.tensor.matmul(A_ps, St[:, :], LT_sb[:, :], start=True, stop=True)
                A_sb = pp.tile([HW, OUT], F32)
                nc.scalar.copy(A_sb, A_ps)
                # U = L @ A  [512, 512], in 4 chunks of 128 rows
                rmax = pp.tile([128, 4], F32)
                for c4 in range(4):
                    U_ps = psm.tile([128, OUT], F32, tag="G")
                    nc.tensor.matmul(U_ps, LT_sb[:, c4 * 128:(c4 + 1) * 128],
                                     A_sb[:, :], start=True, stop=True)
                    U_c = vp.tile([128, OUT], F32, tag="U")
                    nc.scalar.copy(U_c, U_ps)
                    nc.sync.dma_start(
                        out=out_map[c4 * 128:(c4 + 1) * 128, :], in_=U_c)
                    nc.vector.tensor_reduce(out=rmax[:, c4:c4 + 1], in_=U_c,
                                            axis=mybir.AxisListType.X,
                                            op=mybir.AluOpType.max)

                # anomaly_score = max over the upsampled map
                rmax1 = pp.tile([128, 1], F32)
                nc.vector.tensor_reduce(out=rmax1, in_=rmax,
                                        axis=mybir.AxisListType.X,
                                        op=mybir.AluOpType.max)
                nc.sync.dma_start(out=sc_dram[:, :], in_=rmax1)
                rrow = pp.tile([1, 128], F32)
                nc.sync.dma_start(out=rrow,
                                  in_=sc_dram[:, :].rearrange("p a -> a p"))
                s11 = pp.tile([1, 1], F32)
                nc.vector.tensor_reduce(out=s11, in_=rrow,
                                        axis=mybir.AxisListType.X,
                                        op=mybir.AluOpType.max)
                nc.sync.dma_start(out=out_score[:, :], in_=s11)

    nc.finalize()
    return nc


@functools.lru_cache(maxsize=2)
def _get_nc(debug=False):
    return _build(debug)


def _prep_in_maps(feat_map, feature_bank):
    flatT_np = np.ascontiguousarray(feat_map.reshape(C, NPATCH))
    LT_np = np.ascontiguousarray(_resize_matrix(OUT, HW).T)  # [64, 512]
    nsq = (flatT_np.astype(np.float64) ** 2).sum(axis=0)
    rn = (1.0 / np.maximum(np.sqrt(nsq), 1e-12))
    a2 = (nsq * rn * rn).astype(np.float32)
    rn_np = rn.astype(np.float32).reshape(1, NPATCH)
    a2_np = np.ascontiguousarray(a2.reshape(HW, HW).T)
    in_maps = []
    for c in range(N_CORES):
        shard = feature_bank[c * SHARD:(c + 1) * SHARD]          # [6250, 128]
        pad = np.full((PAD_SHARD - SHARD, C), PAD_VAL, np.float32)
        shard = np.concatenate([shard, pad], axis=0)             # [6272, 128]
        shard16 = shard.astype(np.float16)
        bankT_np = np.ascontiguousarray(shard16.T)               # [128, 6272]
        b2 = (shard16.astype(np.float32) ** 2).sum(axis=1)       # [6272]
        b2h_np = (-b2).astype(np.float16).reshape(1, PAD_SHARD)
        b2pp_np = np.ascontiguousarray((-b2).reshape(NQ, 128).T.astype(np.float32))
        in_maps.append({"flatT": flatT_np, "bankT": bankT_np,
                        "b2h": b2h_np, "b2ppi": b2pp_np, "LT": LT_np,
                        "rni": rn_np, "a2i": a2_np})
    return in_maps


def kernel(feat_map, feature_bank, out_size):
    assert int(out_size) == OUT, f"kernel hardcodes out_size={OUT}"
    feat_map = np.asarray(feat_map, dtype=np.float32)
    feature_bank = np.asarray(feature_bank, dtype=np.float32)
    assert feat_map.shape == (1, C, HW, HW)
    assert feature_bank.shape == (BANK, C)

    nc = _get_nc()
    res = run_bass_kernel_spmd(nc, _prep_in_maps(feat_map, feature_bank),
                               core_ids=list(range(N_CORES)))
    r0 = res.results[0]
    anomaly_map = np.asarray(r0["out_map"], dtype=np.float32)
    anomaly_score = np.float32(np.asarray(r0["out_score"]).reshape(()))
    return anomaly_map, anomaly_score
